# revision 10
# baseline (speedup 1.0000x reference)
"""Banded causal self-attention (band width 64) on 8 trn2 NeuronCores.

Sequence-parallel sharding: core c handles batch c//4, query block c%4
(512 queries of T=2048), recomputing a 64-token k/v halo locally so no
collectives are needed. The host casts inputs to bf16 and transposes x
per core; the device kernel fuses qkv-projection -> banded attention ->
output projection.

Device layouts (per core):
  xt    [C, 576]      x chunk transposed (64-token halo + 512 owned)
  qk^T  [2048, 576]   q/k feature-major (slab h//2 (+8 for k), rows (h%2)*64)
  v     [576, 16, 128] token-major, per head [v(64) | ones(64)]
  y^T   [1024, 512]   attention output feature-major
  out   [512, 1024]   tokens x C

Attention is computed transposed (S^T[key, query] per 128-key chunk).
S matmuls are interleaved into the qkv projection so exp (Scalar) and
band-mask multiplies (GpSimd) hide under GEMM time; the masked exp(S^T)
tiles for all 16 heads persist in SBUF. The AV matmul uses a
[v | ones-replicated] stationary so each head's PSUM accumulator holds
yA on rows 0:64 and the softmax denominator replicated on rows 64:128 -
no separate rowsum matmuls and no PSUM zero-init (AV segments split at
first-writer boundaries). Reciprocals run batched on the Scalar engine
(one activation-table swap) and the normalize multiply on Vector.
Pad tokens for the first query block are excluded via a per-core mask
pattern (rows zeroed), so softmax skips max-subtraction and no special
v/ones zeroing is needed.
"""

import numpy as np
import ml_dtypes

import concourse.mybir as mybir
import concourse.tile as tile
from concourse import bacc
from concourse import bass_utils

B, T, C, H, D = 2, 2048, 1024, 16, 64
W = 64            # band width: key j visible to query i iff i-64 <= j <= i
N_CORES = 8
QL = 512          # queries per core
HT = QL + W       # tokens incl. halo
P = 128
KC = C // P       # contraction chunks
NFT = 2 * C // P  # q|k feature slabs
NKC = 5           # key chunks (4x128 + 64)

bf16 = mybir.dt.bfloat16
f32 = mybir.dt.float32
Act = mybir.ActivationFunctionType

_CACHE = {}

# per key-chunk: (chunk keys, query-col start, query-col end, mask pattern)
CHUNKS = []
for c in range(NKC):
    kn = P if c < NKC - 1 else W
    cs = max(0, P * c - W)
    ce = min(QL, P * c + P)
    CHUNKS.append((kn, cs, ce, 0 if c == 0 else 1))

# Pe column offset per chunk (concatenated per-head Pe storage)
PE_OFF = []
_o = 0
for (kn, cs, ce, mi) in CHUNKS:
    PE_OFF.append(_o)
    _o += ce - cs
PE_W = _o  # 896

# AV matmul segments per chunk: (q0, q1, start) split at first-writer
# boundaries so no PSUM region mixes init and accumulate.
AVSEGS = [
    [(0, 128, True)],
    [(64, 128, False), (128, 256, True)],
    [(192, 256, False), (256, 384, True)],
    [(320, 384, False), (384, 512, True)],
    [(448, 512, False)],
]


def _emit(tc, xt, wqk, wv, wp, bqk, bvr, bpr, maskT, out):
    nc = tc.nc
    with (
        tc.tile_pool(name="const", bufs=1) as const,
        tc.tile_pool(name="pet", bufs=4) as pet,
        tc.tile_pool(name="rrp", bufs=2) as rrp,
        tc.tile_pool(name="ot", bufs=3) as ot,
        tc.tile_pool(name="psA", bufs=2, space="PSUM") as psA,
        tc.tile_pool(name="psS", bufs=3, space="PSUM") as psSp,
        tc.tile_pool(name="psY", bufs=3, space="PSUM") as psYp,
    ):
        # ---- persistent tiles ----
        xt_sb = const.tile([P, KC, HT], bf16)
        nc.sync.dma_start(xt_sb[:], xt.rearrange("(kc p) t -> p kc t", p=P))
        wqk_sb = const.tile([P, KC, 2 * C], bf16)
        for j in range(4):
            nc.sync.dma_start(
                wqk_sb[:, :, j * QL:(j + 1) * QL],
                wqk[:, j * QL:(j + 1) * QL].rearrange("(kc p) f -> p kc f", p=P),
            )
        maskT_sb = const.tile([P, 2, P + W], bf16)
        nc.sync.dma_start(maskT_sb[:], maskT.rearrange("m p k -> p m k"))
        bqk_sb = const.tile([P, NFT], f32)
        nc.sync.dma_start(bqk_sb[:], bqk.rearrange("(ft p) -> p ft", p=P))
        bvr_sb = const.tile([P, C], f32)
        nc.sync.dma_start(bvr_sb[:], bvr[:])
        bpr_sb = const.tile([P, C], f32)
        nc.sync.dma_start(bpr_sb[:], bpr[:])
        wv_sb = const.tile([P, KC, C], bf16)
        nc.sync.dma_start(wv_sb[:], wv.rearrange("(kc p) n -> p kc n", p=P))
        wp_sb = const.tile([P, KC, C], bf16)
        nc.sync.dma_start(wp_sb[:], wp.rearrange("(kc p) n -> p kc n", p=P))

        qkT_sb = const.tile([P, NFT, HT], bf16)
        v_sb = const.tile([P, NKC, H, P], bf16)   # per head [v(64) | ones(64)]
        yT_sb = const.tile([P, KC, QL], bf16)
        peA_sb = const.tile([P, H, PE_W], bf16)   # masked exp(S^T), all heads

        # ones columns of the AV stationary (constant; pad exclusion is in
        # the per-core mask)
        nc.gpsimd.memset(v_sb[:, :, :, D:], 1.0)
        zero_sb = const.tile([P, P], bf16)
        nc.gpsimd.memset(zero_sb[:], 0.0)

        # ---- phase 1a: qk^T = Wqk^T @ x^T, S matmuls interleaved ----
        def emit_qk_slab(ft):
            # q is only needed for owned tokens (64:576); k for all 576
            segs = ((W, QL),) if ft < KC else ((0, QL), (QL, W))
            for t0, tsz in segs:
                psf = psA.tile([P, QL], f32, tag="mm", name="ps1a")
                ps = psf[:, :tsz]
                for kc in range(KC):
                    nc.tensor.matmul(
                        ps, wqk_sb[:, kc, ft * P:(ft + 1) * P],
                        xt_sb[:, kc, t0:t0 + tsz],
                        start=(kc == 0), stop=(kc == KC - 1),
                    )
                nc.scalar.activation(
                    qkT_sb[:, ft, t0:t0 + tsz], ps, Act.Identity,
                    bias=bqk_sb[:, ft:ft + 1],
                )

        def emit_scores(hp):
            # Both heads of the pair per chunk: K=64 matmuls at base
            # partitions 0 and 64 land in distinct PE row-groups and run
            # concurrently (tile_position auto-derived).
            for c, (kn, cs, ce, mi) in enumerate(CHUNKS):
                wc = ce - cs
                o = PE_OFF[c]
                pss = [psSp.tile([P, QL], f32, tag="psS", name="psS")
                       for _ in (0, 1)]
                for s in (0, 1):
                    r0 = D * s
                    nc.tensor.matmul(
                        pss[s][:kn, :wc],
                        qkT_sb[r0:r0 + D, KC + hp, c * P:c * P + kn],
                        qkT_sb[r0:r0 + D, hp, W + cs:W + ce],
                        start=True, stop=True,
                    )
                pe = pet.tile([P, 2, 2 * P], bf16, tag="pe", name="pe")
                for s in (0, 1):
                    h = 2 * hp + s
                    nc.scalar.activation(pe[:kn, s, :wc], pss[s][:kn, :wc],
                                         Act.Exp, scale=0.125)
                    nc.vector.tensor_mul(peA_sb[:kn, h, o:o + wc],
                                         pe[:kn, s, :wc],
                                         maskT_sb[:kn, mi, :wc])

        for hp in range(KC):
            emit_qk_slab(hp)         # q slab for heads 2hp, 2hp+1
            emit_qk_slab(KC + hp)    # k slab
            emit_scores(hp)

        # ---- phase 1b: v = x @ Wv (token-major, strided per-head slots) ----
        for tt in range(NKC):
            tsz = P if tt < NKC - 1 else W
            for hb, n0 in ((0, 0), (KC, QL)):
                psf = psA.tile([P, QL], f32, tag="mm", name="ps1b")
                ps = psf[:tsz]
                for kc in range(KC):
                    nc.tensor.matmul(
                        ps, xt_sb[:, kc, tt * P:tt * P + tsz],
                        wv_sb[:, kc, n0:n0 + QL],
                        start=(kc == 0), stop=(kc == KC - 1),
                    )
                nc.vector.tensor_add(
                    v_sb[:tsz, tt, hb:hb + KC, :D],
                    ps.rearrange("p (h e) -> p h e", e=D),
                    bvr_sb[:tsz, n0:n0 + QL].rearrange("p (h e) -> p h e", e=D),
                )

        # ---- phase 2: AV + fused replicated rowsum, normalize ----
        for h in range(H):
            hp, r0 = h // 2, D * (h % 2)
            yA = psYp.tile([P, QL], f32, tag="yA", name="yA")
            nc.tensor.matmul(yA[:], zero_sb[:], xt_sb[:, 0, 0:QL],
                             start=True, stop=False, skip_group_check=True)
            for c, (kn, cs, ce, mi) in enumerate(CHUNKS):
                o = PE_OFF[c]
                nc.tensor.matmul(
                    yA[:, cs:ce],
                    v_sb[:kn, c, h, :],
                    peA_sb[:kn, h, o:o + ce - cs],
                    start=False, stop=(c == NKC - 1),
                    skip_group_check=True,
                )
            rr = rrp.tile([D, QL], f32, tag="rr", name="rr")
            nc.vector.reciprocal(rr[:], yA[D:, :])
            nc.vector.tensor_mul(yT_sb[r0:r0 + D, hp, :], yA[:D, :], rr[:])

        # ---- phase 3: out = y @ Wproj + b ----
        for tt in range(QL // P):
            for n0 in (0, QL):
                ps = psA.tile([P, QL], f32, tag="mm", name="ps3")
                for kc in range(KC):
                    nc.tensor.matmul(
                        ps, yT_sb[:, kc, tt * P:(tt + 1) * P],
                        wp_sb[:, kc, n0:n0 + QL],
                        start=(kc == 0), stop=(kc == KC - 1),
                    )
                osb = ot.tile([P, QL], f32, tag="osb", name="osb")
                nc.vector.tensor_add(osb[:], ps, bpr_sb[:, n0:n0 + QL])
                nc.sync.dma_start(out[tt * P:(tt + 1) * P, n0:n0 + QL], osb[:])


def _build():
    nc = bacc.Bacc(
        "TRN2", target_bir_lowering=False, debug=False,
        enable_asserts=True, num_devices=N_CORES,
    )
    xt = nc.dram_tensor("xt", [C, HT], bf16, kind="ExternalInput").ap()
    wqk = nc.dram_tensor("wqk", [C, 2 * C], bf16, kind="ExternalInput").ap()
    wv = nc.dram_tensor("wv", [C, C], bf16, kind="ExternalInput").ap()
    wp = nc.dram_tensor("wp", [C, C], bf16, kind="ExternalInput").ap()
    bqk = nc.dram_tensor("bqk", [2 * C], f32, kind="ExternalInput").ap()
    bvr = nc.dram_tensor("bvr", [P, C], f32, kind="ExternalInput").ap()
    bpr = nc.dram_tensor("bpr", [P, C], f32, kind="ExternalInput").ap()
    maskT = nc.dram_tensor("maskT", [2, P, P + W], bf16, kind="ExternalInput").ap()
    out = nc.dram_tensor("out", [QL, C], f32, kind="ExternalOutput").ap()
    with tile.TileContext(nc) as tc:
        _emit(tc, xt, wqk, wv, wp, bqk, bvr, bpr, maskT, out)
    nc.compile()
    return nc


def _get_module():
    if "nc" not in _CACHE:
        _CACHE["nc"] = _build()
    return _CACHE["nc"]


def _band_masks(pad_first: bool) -> np.ndarray:
    # pattern 0 (chunk 0):  keep iff  y <= p <= y+64
    # pattern 1 (chunks>0): keep iff  y-64 <= p <= y
    p = np.arange(P)[:, None]
    y = np.arange(P + W)[None, :]
    m0 = (p >= y) & (p <= y + W)
    m1 = (p >= y - W) & (p <= y)
    m = np.stack([m0, m1]).astype(ml_dtypes.bfloat16)
    if pad_first:
        # first query block: halo keys 0:64 are zero-pad tokens
        m[0, :W, :] = 0
    return m


def kernel(x, Wqkv, bqkv, Wproj, bproj):
    x = np.asarray(x, dtype=np.float32)
    Wqkv = np.asarray(Wqkv, dtype=np.float32)
    bqkv = np.asarray(bqkv, dtype=np.float32)
    Wproj = np.asarray(Wproj, dtype=np.float32)
    bproj = np.asarray(bproj, dtype=np.float32)

    bf = ml_dtypes.bfloat16
    wqk_np = np.ascontiguousarray(Wqkv[:, :2 * C]).astype(bf)
    wv_np = np.ascontiguousarray(Wqkv[:, 2 * C:]).astype(bf)
    wp_np = Wproj.astype(bf)
    bqk_np = np.ascontiguousarray(bqkv[:2 * C])
    bvr_np = np.ascontiguousarray(np.broadcast_to(bqkv[2 * C:], (P, C)))
    bpr_np = np.ascontiguousarray(np.broadcast_to(bproj, (P, C)))
    mask_np = _band_masks(False)
    mask0_np = _band_masks(True)

    in_maps = []
    for c in range(N_CORES):
        b, q = divmod(c, 4)
        lo = q * QL - W
        if lo < 0:
            chunk = np.concatenate(
                [np.zeros((W, C), np.float32), x[b, 0:q * QL + QL]], axis=0
            )
        else:
            chunk = x[b, lo:lo + HT]
        in_maps.append({
            "xt": np.ascontiguousarray(chunk.T).astype(bf),
            "wqk": wqk_np,
            "wv": wv_np,
            "wp": wp_np,
            "bqk": bqk_np,
            "bvr": bvr_np,
            "bpr": bpr_np,
            "maskT": mask0_np if q == 0 else mask_np,
        })

    nc = _get_module()
    _CACHE["last_in_maps"] = in_maps
    res = bass_utils.run_bass_kernel_spmd(nc, in_maps, core_ids=list(range(N_CORES)))

    out = np.empty((B, T, C), dtype=np.float32)
    for c in range(N_CORES):
        b, q = divmod(c, 4)
        out[b, q * QL:(q + 1) * QL] = res.results[c]["out"]
    return out
